# revision 1
# baseline (speedup 1.0000x reference)
"""CFConv (SchNet continuous-filter conv) TRN2 Bass kernel, 8-core row-parallel.

Reference computation (batch=1):
  w1 = silu(e @ w1_w^T + w1_b)        (512, 511, 128)
  w2 = silu(e @ w2_w^T + w2_b)
  xp = silu(x @ phi_w^T + phi_b)      (512, 128)
  x_nbs[i, j] = xp[j + (j >= i)]      neighbor gather
  v  = (concat(xp_i * w1, x_nbs * w2) @ o_w^T + o_b) * mask
  return split(v, 3, axis=-1)

Strategy (SPMD, 64 atom-rows per core, no collectives):
  - host pre-scatters e/mask into atom-indexed (512-wide) layout with the
    diagonal zeroed, so the neighbor gather becomes plain elementwise vs xp^T
    and the program is identical across cores.  The inner *mask on v1/v2 in
    the reference is redundant (mask is {0,1} and is re-applied after the
    per-edge matmul), so a single final mask suffices.
  - host pre-transposes every operand into its on-chip layout (features/k on
    partitions), so the device does zero transposes:
      eT[g, 32r+k, n] = e[row 4g+r, atom n, k]  -- the 4 rows of a group sit
      at partition bases 0/32/64/96, matching PE row-group tile_position for
      the K=20 filter matmuls; plus maskT, xT, xoT, phi_wT, w{1,2}T_rep
      (replicated at the 4 bases), o_wT1/o_wT2 (the two K=128 halves of
      o_w^T), ob_rep, ones.
  - per row: w1^T/w2^T (128f x 512 atoms) = wT_rep @ eT slice (PE, fp32r);
    silu on ACT (psum->sbuf, bias fused); v1^T = w1sil * xp_own[:, row]
    (DVE per-partition scalar); v2^T = w2sil * xp^T (DVE elementwise).
  - per atom-tile t: PSUM (128, 384) accumulates a K=1 bias matmul
    (ones ⊗ o_b, concurrent across the 4 PE row-groups) + v1T^T @ o_wT1 +
    v2T^T @ o_wT2; the epilogue scales by mask (per-partition scalar) on the
    PSUM->SBUF pass, split between ACT and DVE; one 786KB DMA per row.
  - host gathers the 512-wide atom-indexed output back to 511-wide and
    splits into the (s1, s2, s3) tuple.

Matmul dtype float32r (fp32 with 11-bit mantissa, 1 cyc/row at N>=256;
end-to-end max rel err ~3e-4); MM_DT="f32" switches to exact fp32
(4 cyc/row, ~3x slower) if tighter precision is required.

Cost-model timeline sim: ~170 us/core (DMA 155 us and PE 155 us co-binding;
output writes 50 MB/core are the memory roofline ~145 us at 358 GB/s).
The epilogue alternates ACT/DVE per tile and each row's output goes out as
two half-row DMAs so the store overlaps the remaining epilogue tiles.
"""

import sys

sys.path.insert(0, "/opt/trn_rl_repo")

import numpy as np  # noqa: E402

N_A, N_F, N_K, N_G = 512, 128, 20, 384
CORES = 8
ROWS = N_A // CORES          # 64 rows per core
R = 4                        # rows per transpose group
NGRP = ROWS // R
NT = N_A // 128              # 4 atom tiles

MM_DT = "f32r"               # "f32r" | "f32"
V2_ENGINE = "dve"            # "dve" | "pool"
ETCOPY_ENGINE = "dve"        # "act" | "dve"
EPI_ACT_TILES = 2            # how many of the 4 out-tiles ACT handles
BIAS_CONC = True            # concurrent K=1 bias matmuls via row groups

_STATE = {}


def _build_nc():
    import concourse.bacc as bacc
    import concourse.mybir as mybir
    import concourse.tile as tile
    from concourse import masks

    F32 = mybir.dt.float32
    MMD = mybir.dt.float32r if MM_DT == "f32r" else mybir.dt.float32
    Silu = mybir.ActivationFunctionType.Silu
    Copy = mybir.ActivationFunctionType.Copy
    MUL = mybir.AluOpType.mult

    nc = bacc.Bacc(None)

    d_xT = nc.dram_tensor("xT", [N_F, N_A], F32, kind="ExternalInput")
    d_xoT = nc.dram_tensor("xoT", [N_F, ROWS], F32, kind="ExternalInput")
    d_e = nc.dram_tensor("eT", [NGRP, 128, N_A], F32, kind="ExternalInput")
    d_m = nc.dram_tensor("maskT", [128, NT * ROWS], F32, kind="ExternalInput")
    d_w1T = nc.dram_tensor("w1T_rep", [128, 128], F32, kind="ExternalInput")
    d_w2T = nc.dram_tensor("w2T_rep", [128, 128], F32, kind="ExternalInput")
    d_w1b = nc.dram_tensor("w1_b", [N_F, 1], F32, kind="ExternalInput")
    d_w2b = nc.dram_tensor("w2_b", [N_F, 1], F32, kind="ExternalInput")
    d_pwT = nc.dram_tensor("phi_wT", [N_F, N_F], F32, kind="ExternalInput")
    d_pb = nc.dram_tensor("phi_b", [N_F, 1], F32, kind="ExternalInput")
    d_oT1 = nc.dram_tensor("o_wT1", [N_F, N_G], F32, kind="ExternalInput")
    d_oT2 = nc.dram_tensor("o_wT2", [N_F, N_G], F32, kind="ExternalInput")
    d_obr = nc.dram_tensor("ob_rep", [128, N_G], F32, kind="ExternalInput")
    d_ones = nc.dram_tensor("ones", [128, 128], F32, kind="ExternalInput")
    d_out = nc.dram_tensor("out", [ROWS, N_A, N_G], F32, kind="ExternalOutput")

    with tile.TileContext(nc) as tc:
        with tc.tile_pool(name="static", bufs=1) as st:
            # ---- static loads (already transposed on host) ----
            maskT = st.tile([128, NT * ROWS], F32)
            nc.sync.dma_start(maskT[:], d_m[:])
            xT = st.tile([N_F, N_A], F32)
            nc.sync.dma_start(xT[:], d_xT[:])
            xoT = st.tile([N_F, ROWS], F32)
            nc.sync.dma_start(xoT[:], d_xoT[:])
            phi_wT = st.tile([N_F, N_F], F32)
            nc.sync.dma_start(phi_wT[:], d_pwT[:])
            w1Tf = st.tile([128, 128], F32)
            nc.sync.dma_start(w1Tf[:], d_w1T[:])
            w2Tf = st.tile([128, 128], F32)
            nc.sync.dma_start(w2Tf[:], d_w2T[:])
            oT1f = st.tile([N_F, N_G], F32)
            nc.sync.dma_start(oT1f[:], d_oT1[:])
            oT2f = st.tile([N_F, N_G], F32)
            nc.sync.dma_start(oT2f[:], d_oT2[:])
            obf = st.tile([128, N_G], F32)
            nc.sync.dma_start(obf[:], d_obr[:])
            onesf = st.tile([128, 128], F32)
            nc.sync.dma_start(onesf[:], d_ones[:])
            w1b = st.tile([N_F, 1], F32)
            nc.sync.dma_start(w1b[:], d_w1b[:])
            w2b = st.tile([N_F, 1], F32)
            nc.sync.dma_start(w2b[:], d_w2b[:])
            pb = st.tile([N_F, 1], F32)
            nc.sync.dma_start(pb[:], d_pb[:])

            # ---- static casts + one-time xp matmuls ----
            w1rep = st.tile([128, 128], MMD)
            w2rep = st.tile([128, 128], MMD)
            o_wT1 = st.tile([N_F, N_G], MMD)
            o_wT2 = st.tile([N_F, N_G], MMD)
            ob_r = st.tile([128, N_G], MMD)
            ones_r = st.tile([128, 128], MMD)
            maskTT = maskT
            xpT = st.tile([N_F, N_A], F32)
            xpTo = st.tile([N_F, ROWS], F32)
            with tc.tile_pool(name="ips", bufs=2, space="PSUM") as ips:
                nc.vector.tensor_copy(w1rep[:], w1Tf[:])
                nc.vector.tensor_copy(w2rep[:], w2Tf[:])
                nc.vector.tensor_copy(o_wT1[:], oT1f[:])
                nc.vector.tensor_copy(o_wT2[:], oT2f[:])
                nc.vector.tensor_copy(ob_r[:], obf[:])
                nc.vector.tensor_copy(ones_r[:], onesf[:])
                p = ips.tile([128, N_A], F32, tag="ip")
                nc.tensor.matmul(p[:], phi_wT[:], xT[:], start=True, stop=True)
                nc.scalar.activation(xpT[:], p[:], Silu, bias=pb[:])
                p = ips.tile([128, N_A], F32, tag="ip")
                nc.tensor.matmul(p[:, 0:ROWS], phi_wT[:], xoT[:],
                                 start=True, stop=True)
                nc.scalar.activation(xpTo[:], p[:, 0:ROWS], Silu, bias=pb[:])

            with tc.tile_pool(name="loop", bufs=1) as lp, \
                 tc.tile_pool(name="wps", bufs=5, space="PSUM") as wps, \
                 tc.tile_pool(name="ops", bufs=3, space="PSUM") as ops:

                def load_group(g):
                    eTf = lp.tile([128, N_A], F32, tag="eTf", bufs=4)
                    nc.sync.dma_start(eTf[:], d_e[g])
                    eT4 = lp.tile([128, N_A], MMD, tag="eT4", bufs=4)
                    nc.vector.tensor_copy(eT4[:], eTf[:])
                    return eT4

                eT4s = {0: load_group(0), 1: load_group(1)}

                for g in range(NGRP):
                    eT4 = eT4s.pop(g)
                    if g + 2 < NGRP:
                        eT4s[g + 2] = load_group(g + 2)
                    for r in range(R):
                        i = R * g + r            # row within this core's shard
                        w1ps = wps.tile([128, N_A], F32, tag="wps")
                        nc.tensor.matmul(w1ps[:], w1rep[32 * r:32 * r + N_K, :],
                                         eT4[32 * r:32 * r + N_K, :],
                                         start=True, stop=True,
                                         tile_position=(32 * r, 0))
                        w2ps = wps.tile([128, N_A], F32, tag="wps")
                        nc.tensor.matmul(w2ps[:], w2rep[32 * r:32 * r + N_K, :],
                                         eT4[32 * r:32 * r + N_K, :],
                                         start=True, stop=True,
                                         tile_position=(32 * r, 0))
                        w1sil = lp.tile([128, N_A], F32, tag="w1sil", bufs=2)
                        nc.scalar.activation(w1sil[:], w1ps[:], Silu, bias=w1b[:])
                        w2sil = lp.tile([128, N_A], F32, tag="w2sil", bufs=2)
                        nc.scalar.activation(w2sil[:], w2ps[:], Silu, bias=w2b[:])
                        v2T = lp.tile([128, N_A], MMD, tag="v2T", bufs=2)
                        if V2_ENGINE == "dve":
                            nc.vector.tensor_tensor(v2T[:], w2sil[:], xpT[:], MUL)
                        else:
                            nc.gpsimd.tensor_tensor(v2T[:], w2sil[:], xpT[:], MUL)
                        v1T = lp.tile([128, N_A], MMD, tag="v1T", bufs=2)
                        nc.vector.tensor_scalar_mul(v1T[:], w1sil[:],
                                                    xpTo[:, i:i + 1])
                        out_sb = lp.tile([128, NT * N_G], F32, tag="out_sb",
                                         bufs=8)
                        ops_t = []
                        for t in range(NT):
                            op = ops.tile([128, N_G], F32, tag="op")
                            ops_t.append(op)
                            if BIAS_CONC:
                                nc.tensor.matmul(op[:], ones_r[32 * t:32 * t + 1, :],
                                                 ob_r[32 * t:32 * t + 1, :],
                                                 start=True, stop=False,
                                                 tile_position=(32 * t, 0))
                            else:
                                nc.tensor.matmul(op[:], ones_r[0:1, :], ob_r[0:1, :],
                                                 start=True, stop=False)
                        for t in range(NT):
                            op = ops_t[t]
                            nc.tensor.matmul(op[:], v1T[:, 128 * t:128 * (t + 1)],
                                             o_wT1[:], start=False, stop=False)
                            nc.tensor.matmul(op[:], v2T[:, 128 * t:128 * (t + 1)],
                                             o_wT2[:], start=False, stop=True)
                            mcol = maskT[:, t * ROWS + i:t * ROWS + i + 1]
                            dst = out_sb[:, t * N_G:(t + 1) * N_G]
                            if t % 2 == 0:
                                nc.scalar.activation(dst, op[:], Copy, scale=mcol)
                            else:
                                nc.vector.tensor_scalar_mul(dst, op[:], mcol)
                            if t == 1:
                                nc.sync.dma_start(
                                    d_out[i, 0:256, :].rearrange(
                                        "(t p) g -> p t g", p=128),
                                    out_sb[:, 0:2 * N_G].rearrange(
                                        "p (t g) -> p t g", g=N_G))
                            elif t == 3:
                                nc.sync.dma_start(
                                    d_out[i, 256:512, :].rearrange(
                                        "(t p) g -> p t g", p=128),
                                    out_sb[:, 2 * N_G:4 * N_G].rearrange(
                                        "p (t g) -> p t g", g=N_G))


    nc.compile()
    return nc


def _get_state():
    if "nc" not in _STATE:
        _STATE["nc"] = _build_nc()
        # pos->atom index map per core: a = j + (j >= i_abs)
        j = np.arange(N_A - 1)[None, :]
        scat = []
        for c in range(CORES):
            i_abs = (c * ROWS + np.arange(ROWS))[:, None]
            scat.append((j + (j >= i_abs)).astype(np.int64))  # (ROWS, 511)
        _STATE["aidx"] = scat
        _STATE["rows"] = np.arange(ROWS)[:, None]
    return _STATE


def build_in_maps(x, e, mask, w1_w, w1_b, w2_w, w2_b, phi_w, phi_b, o_w, o_b):
    """x (512,128), e (512,511,20), mask (512,511) fp32 -> per-core in_maps."""
    st = _get_state()
    rows = st["rows"]

    def _rep4(wT):  # (20,128) -> (128,128) at partition bases 0/32/64/96
        out = np.zeros((128, wT.shape[1]), np.float32)
        for r in range(4):
            out[32 * r:32 * r + wT.shape[0]] = wT
        return out

    x = np.asarray(x, np.float32)
    o_wn = np.asarray(o_w, np.float32)             # (384, 256)
    shared = {
        "xT": np.ascontiguousarray(x.T),                             # (128,512)
        "phi_wT": np.ascontiguousarray(np.asarray(phi_w, np.float32).T),
        "phi_b": np.asarray(phi_b, np.float32).reshape(N_F, 1),
        "w1_b": np.asarray(w1_b, np.float32).reshape(N_F, 1),
        "w2_b": np.asarray(w2_b, np.float32).reshape(N_F, 1),
        "w1T_rep": _rep4(np.asarray(w1_w, np.float32).T),
        "w2T_rep": _rep4(np.asarray(w2_w, np.float32).T),
        "o_wT1": np.ascontiguousarray(o_wn[:, 0:N_F].T),             # (128,384)
        "o_wT2": np.ascontiguousarray(o_wn[:, N_F:2 * N_F].T),
        "ob_rep": np.ascontiguousarray(
            np.broadcast_to(np.asarray(o_b, np.float32).reshape(1, N_G),
                            (128, N_G))),
        "ones": np.ones((128, 128), np.float32),
    }

    in_maps = []
    for c in range(CORES):
        sl = slice(c * ROWS, (c + 1) * ROWS)
        aidx = st["aidx"][c]
        e_at = np.zeros((ROWS, N_A, N_K), np.float32)
        e_at[rows, aidx] = e[sl]
        # eT[g, 32r+k, n] = e_at[4g+r, n, k]
        eT = np.zeros((NGRP, 128, N_A), np.float32)
        eT.reshape(NGRP, R, 32, N_A)[:, :, 0:N_K] = \
            e_at.reshape(NGRP, R, N_A, N_K).transpose(0, 1, 3, 2)
        m_at = np.zeros((ROWS, N_A), np.float32)
        m_at[rows, aidx] = mask[sl]
        # maskT[p, t*64+i] = m_at[i, 128t+p]
        mT = np.ascontiguousarray(
            m_at.reshape(ROWS, NT, 128).transpose(2, 1, 0).reshape(128, NT * ROWS))
        in_maps.append({
            "eT": eT, "maskT": mT,
            "xoT": np.ascontiguousarray(x[sl].T),                    # (128, 64)
            **shared,
        })
    return in_maps


def kernel(x, e, mask, loop_mask, w1_w, w1_b, w2_w, w2_b, phi_w, phi_b, o_w, o_b):
    st = _get_state()
    from concourse.bass_utils import run_bass_kernel_spmd

    x = np.asarray(x, np.float32)[0]                                # (512,128)
    e = np.asarray(e, np.float32)[0]                                # (512,511,20)
    mask = np.asarray(mask, np.float32)[0, :, :, 0]                 # (512,511)
    in_maps = build_in_maps(x, e, mask, w1_w, w1_b, w2_w, w2_b,
                            phi_w, phi_b, o_w, o_b)

    res = run_bass_kernel_spmd(st["nc"], in_maps, list(range(CORES)))

    rows = st["rows"]
    parts = []
    for c in range(CORES):
        out_at = res.results[c]["out"]                  # (ROWS, 512, 384)
        parts.append(out_at[rows, st["aidx"][c]])       # (ROWS, 511, 384)
    v = np.concatenate(parts, axis=0)[None]             # (1, 512, 511, 384)
    s1, s2, s3 = np.split(v, 3, axis=-1)
    return (s1, s2, s3)

